# revision 11
# baseline (speedup 1.0000x reference)
"""Always-on MoE forward (expert 0 dense + top-k of 7 routed) on 8 TRN2 cores.

Strategy
--------
The router (4096x1024 @ 1024x7 matmul + softmax + top-2) is ~58 MFLOP --
negligible -- so it runs on host in numpy as part of computing the sharding
plan.  The expensive part (expert SwiGLU MLPs, ~155 GFLOP with top-2
sparsity) runs on device, expert-parallel with host-side token
dispatch/combine:

- SPMD graph: every core runs TWO or THREE weight groups with compile-time
  capacities.  Each (core, group) slot is filled with tokens of ONE expert
  (weights supplied per-core via in_maps).  A capacity search (2-group
  legacy + 3-group DFS assignment where an expert may span several slots on
  different cores) picks the group sizes minimizing the modeled PE-stream
  time; with 3 groups the per-core capacity lands within ~1% of the ideal
  (T + k*T) / 8 balance point (e.g. caps (344, 412, 792) -> 1548 vs 1536
  ideal, vs 1586 for the best 2-group plan).  Leftover slots are filled
  with always-on expert-0 tokens.
- Host gathers each core's tokens (transposed k-chunk layout, bf16), device
  computes down(silu(x@wg) * (x@wu)) for both groups, host scatter-adds the
  outputs with the combine weights (expert-0 weight 1.0).

Device kernel: pure dense matmul streaming, weights stationary in SBUF per
group.  All DRAM views are pre-tiled on host into [128, flat] layouts so
every DMA is contiguous per partition and every matmul operand is a direct
SBUF slice; weights are DMA'd per 128-wide m-tile (0.25 MB) so the first
matmuls start ~8 us into the kernel instead of waiting for full weight sets,
and a PE pre-warm burst flips the HAM clock gate during the DMA ramp.
"""

import numpy as np
import ml_dtypes

D = 1024
DFF = 2048
E = 8
NCORES = 8
T = 2 * 2048  # B * S
KD = D // 128    # contraction chunks over D
KF = DFF // 128  # contraction chunks over DFF

_COMPILED = {}

_BF16 = ml_dtypes.bfloat16

LAST_EXEC_NS = None
LAST_RESULT = None


def _route(x, router_w, router_b, top_k):
    """Replicates the reference router in numpy f32: returns (topi, topw)."""
    logits = x.astype(np.float32) @ router_w.astype(np.float32) + router_b.astype(
        np.float32
    )
    m = logits.max(axis=-1, keepdims=True)
    p = np.exp(logits - m)
    p /= p.sum(axis=-1, keepdims=True)
    k = int(top_k)
    topi = np.argpartition(-p, kth=k - 1, axis=-1)[:, :k]  # top-k set (unordered)
    topw = np.take_along_axis(p, topi, axis=-1)
    topw = topw / topw.sum(axis=-1, keepdims=True)
    return topi, topw.astype(np.float32)


def _split_even(idx, w, n):
    """Split (idx, w) into n near-equal chunks."""
    c = len(idx)
    sizes = [(c + n - 1 - i) // n for i in range(n)]
    out, pos = [], 0
    for s in sizes:
        out.append((idx[pos : pos + s], w[pos : pos + s]))
        pos += s
    return out


def _widths_for(cap):
    """Tile widths for a group capacity.

    First tile as wide as possible (512) -- during the first tile the weight
    stream runs near the HBM rate, and a wider tile lowers the per-ns weight
    demand.  Remaining capacity in near-equal tiles, kept above the ~280
    matmul instruction floor (LDWEIGHTS + dispatch) when possible.
    """
    if cap <= 0:
        return []

    def near_equal(c, n):
        return [(c + n - 1 - i) // n for i in range(n)]

    plain = near_equal(cap, -(-cap // 512))
    if cap > 512:
        rem = cap - 512
        lead = [512] + near_equal(rem, -(-rem // 512))
    else:
        lead = plain
    cost = lambda ws: sum(max(w, 280) for w in ws)
    return lead if cost(lead) <= cost(plain) else plain


def _tile_cost(cap):
    """Modeled PE-stream ns for one group: 384 matmuls per tile, each
    max(streaming, LDWEIGHTS-floor ~108ns)."""
    return sum(384 * max(w / 2.4 + 2.5, 108.0) for w in _widths_for(cap))


def _assign_k3(caps, counts_list):
    """DFS: assign each routed expert a (i0,i1,i2) slot combo (i_g slots of
    group g); e0 must fit in the leftover slots.  Cost is fixed by `caps`,
    so the FIRST feasible assignment wins (min-capacity-first ordering keeps
    leftover for e0 large).  Returns {expert: (i0,i1,i2)} or None."""
    sizes = caps
    total_cap = NCORES * sum(sizes)
    order = sorted(counts_list, key=lambda ec: -ec[1])
    combos_per = []
    for e, c in order:
        opts = []
        for i0 in range(4):
            for i1 in range(4):
                for i2 in range(4):
                    capx = i0 * sizes[0] + i1 * sizes[1] + i2 * sizes[2]
                    if capx >= c:
                        opts.append((capx, (i0, i1, i2)))
        opts.sort()
        if not opts:
            return None
        combos_per.append((e, c, opts[:10]))
    # lower bound on remaining assigned capacity from expert idx onward
    lb = [0] * (len(combos_per) + 1)
    for i in range(len(combos_per) - 1, -1, -1):
        lb[i] = lb[i + 1] + combos_per[i][2][0][0]
    found = [None]
    budget = [60000]
    def dfs(idx, use, tot, picks):
        if found[0] is not None or budget[0] <= 0:
            return
        budget[0] -= 1
        if total_cap - tot - lb[idx] < T:
            return  # e0 can no longer fit
        if idx == len(combos_per):
            found[0] = dict(picks)
            return
        e, c, opts = combos_per[idx]
        for capx, inc in opts:
            nu = [u + i for u, i in zip(use, inc)]
            if any(u > NCORES for u in nu):
                continue
            picks.append((e, inc))
            dfs(idx + 1, nu, tot + capx, picks)
            picks.pop()
            if found[0] is not None:
                return
    dfs(0, [0, 0, 0], 0, [])
    return found[0]


def _plan_k2(counts):
    """Legacy 2-group plan: (cost, caps, combos) -- each routed expert gets
    j B-slots; e0 fills the rest."""
    cands = sorted(
        {-(-c // j) for c in counts.values() for j in range(1, 9) if c} | {512}
    )
    best = None
    for Bc in cands:
        npieces = sum(-(-c // Bc) for c in counts.values() if c)
        if npieces > NCORES:
            continue
        nfree = NCORES - npieces
        e0_in_b = min(T, nfree * Bc)
        A = -(-(T - e0_in_b) // NCORES) if e0_in_b < T else 0
        tot = _tile_cost(A) + _tile_cost(Bc)
        if best is None or tot < best[0]:
            best = (tot, A, Bc)
    tot, A, B = best
    combos = {e: (0, -(-c // B)) for e, c in counts.items() if c}
    return tot, (A, B), combos


def _plan_k3(counts):
    """Search 3-group capacities; returns (cost, caps, combos) or None."""
    from functools import lru_cache

    tc = lru_cache(maxsize=None)(_tile_cost)
    lo = -(-(T + sum(counts.values())) // NCORES)
    counts_list = [(e, c) for e, c in counts.items() if c > 0]
    cands = []
    for C in range(560, 1241, 4):
        cC = tc(C)
        for B in range(240, C + 1, 4):
            cB = tc(B)
            a0 = max(160, lo - B - C)
            a1 = min(B, lo + 52 - B - C)
            a0 += (-a0) % 4
            for A in range(a0, a1 + 1, 4):
                cands.append((tc(A) + cB + cC, A, B, C))
    cands.sort()
    tried = 0
    for cost, A, B, C in cands:
        tried += 1
        if tried > 25000:
            break
        picks = _assign_k3((A, B, C), counts_list)
        if picks is not None:
            return cost, (A, B, C), picks
    return None


def _plan_slots(per_expert):
    """Pick group capacities and fill the 8 slots per group.

    per_expert: {e: (idx, w)} for routed experts.
    Returns (caps, slots); slots[c] = tuple over groups of (e, idx, w).
    """
    counts = {e: len(v[0]) for e, v in per_expert.items()}
    cost2, caps2, combos2 = _plan_k2(counts)
    plan3 = _plan_k3(counts)
    if plan3 is not None and plan3[0] < cost2 - 2000:
        cost, caps, combos = plan3
    else:
        cost, caps, combos = cost2, caps2, combos2

    ngroups = len(caps)
    ones = np.ones(T, dtype=np.float32)
    e0_idx = np.arange(T, dtype=np.int64)

    # Build expert pieces per group: fill assigned slots, remainder last.
    group_pieces = [[] for _ in range(ngroups)]
    for e in sorted(counts, key=lambda e: -counts[e]):
        if counts[e] == 0:
            continue
        idx, w = per_expert[e]
        combo = combos[e]
        slot_sizes = []
        for g in range(ngroups):
            slot_sizes += [(g, caps[g])] * combo[g]
        # fill largest slots first so at most one slot is partial
        slot_sizes.sort(key=lambda gs: -gs[1])
        pos = 0
        for g, size in slot_sizes:
            take = min(size, len(idx) - pos)
            group_pieces[g].append((e, idx[pos : pos + take], w[pos : pos + take]))
            pos += take
        assert pos >= len(idx), f"expert {e} tokens not covered"
    # e0 fills remaining slots (largest first so at most one is partial)
    free = []
    for g in range(ngroups):
        free += [(g, caps[g])] * (NCORES - len(group_pieces[g]))
    free.sort(key=lambda gs: -gs[1])
    pos = 0
    for g, size in free:
        take = min(size, T - pos)
        group_pieces[g].append((0, e0_idx[pos : pos + take], ones[pos : pos + take]))
        pos += take
    assert pos >= T, "expert-0 tokens not covered"

    slots = [
        tuple(group_pieces[g][c] for g in range(ngroups)) for c in range(NCORES)
    ]
    return caps, slots


def _wgu_layout(w2d):
    """[D, DFF] f32 -> [128, KF*KD*128] bf16 m-tile-major layout:
    element [p, (m*KD + k)*128 + c] = W[k*128 + p, m*128 + c]."""
    a = w2d.reshape(KD, 128, KF, 128).transpose(1, 2, 0, 3).reshape(128, -1)
    return np.ascontiguousarray(a).astype(_BF16)


def _wd_layout(w2d):
    """[DFF, D] f32 -> [128, KD*KF*128] bf16 m-tile-major layout:
    element [p, (m*KF + k)*128 + c] = W[k*128 + p, m*128 + c]."""
    a = w2d.reshape(KF, 128, KD, 128).transpose(1, 2, 0, 3).reshape(128, -1)
    return np.ascontiguousarray(a).astype(_BF16)


def _tiles_for(caps):
    """Token tiles [(group, start_in_group, width, flat_offset)] per core."""
    tiles = []
    off = 0
    for g, cap in enumerate(caps):
        ts = 0
        for w in _widths_for(cap):
            tiles.append((g, ts, w, off))
            off += w
            ts += w
    return tiles, off


def _build_graph(caps):
    import concourse.mybir as mybir
    import concourse.tile as tile
    from concourse import bacc
    from contextlib import ExitStack

    bf16 = mybir.dt.bfloat16
    f32 = mybir.dt.float32

    tiles, total = _tiles_for(caps)
    groups = [g for g in range(len(caps)) if caps[g] > 0]

    nc = bacc.Bacc("TRN2", target_bir_lowering=False)

    xt_d = nc.declare_dram_parameter("xt", [128, KD * total], bf16, isOutput=False)
    w_ds = []
    for g in groups:
        w_ds.append(
            (
                nc.declare_dram_parameter(
                    f"w{g}g", [128, KF * KD * 128], bf16, isOutput=False
                ),
                nc.declare_dram_parameter(
                    f"w{g}u", [128, KF * KD * 128], bf16, isOutput=False
                ),
                nc.declare_dram_parameter(
                    f"w{g}d", [128, KD * KF * 128], bf16, isOutput=False
                ),
            )
        )
    out_d = nc.declare_dram_parameter("out", [128, KD * total], bf16, isOutput=True)

    with tile.TileContext(nc) as tc, ExitStack() as ctx:
        # One SBUF pool + one PSUM pool (per-tag bufs) -- the end-of-kernel
        # semaphore drain scales with pool count, so fewer pools shorten the
        # epilogue.
        sbp = ctx.enter_context(tc.tile_pool(name="sb", bufs=1))
        wpool = xpool = hpool = gpool = opool = sbp
        psp = ctx.enter_context(tc.tile_pool(name="ps", bufs=2, space="PSUM"))
        psg = psu = psd = psp

        GU_M = KD * 128  # bytes-per-m-tile span (elems) for wg/wu
        D_M = KF * 128   # for wd

        # PE pre-warm: a burst of dummy matmuls that (a) bridges the gap
        # until the first weight/token chunks land (~11 us: preamble ~7.4 +
        # DMA cold latency) and (b) accumulates the ~3.4 us of sustained PE
        # busy the HAM clock gate needs to flip 4/8 -> 8/8, so the first
        # REAL matmuls run at 2.4 GHz instead of 1.2.  38 x ~107 ns cold
        # then ~56 ns once warm lands the burst end at ~11.5 us.
        warm_sb = wpool.tile([128, 128], bf16, tag="warm")
        nc.gpsimd.memset(warm_sb[:], 0)
        ps_w = psp.tile([128, 128], f32, tag="psw", bufs=1)
        for _ in range(38):
            nc.tensor.matmul(ps_w[:], warm_sb[:], warm_sb[:], start=True, stop=True)

        # Larger group first: its weights stream in unblocked at t=0, and the
        # other group's weight reloads (WAR-gated on this group's last use of
        # each m-tile slot) get a long compute window to hide under.
        order = sorted(range(len(groups)), key=lambda gi: -caps[groups[gi]])

        # DMA choreography (v3): the head of the kernel is HBM-contention
        # bound (all 8 cores pull their lead blocks at once), so each of the
        # three demand streams gets its own ring, sized to its per-m demand
        # rate (~75 GB/s each for wg and wu with the interleaved gate/up
        # loop):
        #   sync  (SP ring) : wg geometric blocks, then wd blocks, per group
        #   scalar(Act ring): xt head chunks + per-tile xt prefetch + output
        #                     DMAs (issue order between silu sections paces
        #                     them off the contended head)
        #   gpsimd (SWDGE)  : wu geometric blocks (~2 us latency is fine --
        #                     first use is +1.7 us after gate m0, and the
        #                     ring has nothing else on it)
        # Few, geometric blocks: the HWDGE issue path allows only ~6
        # outstanding dma_starts per engine, and the Act ring must stay
        # clear so silu ACTIVATEs are never queued behind DMA issues.
        GU_BLKS = [(0, 1), (1, 2), (2, 4), (4, 8), (8, 16)]   # m-tile ranges
        WD_BLKS = [(0, 4), (4, 8)]                            # m2-tile ranges

        # Global tile list in execution order (groups ordered, tiles within)
        exec_tiles = []
        for gi in order:
            g = groups[gi]
            for t in tiles:
                if t[0] == g:
                    exec_tiles.append(t)

        # xt SBUF tiles, created lazily; bufs=3 paces the prefetch (tile
        # i+3's DMA WAR-waits on tile i's last read, keeping its bytes off
        # the head-critical window).
        xt_sbs = {}

        def make_xt(i):
            if i in xt_sbs or i >= len(exec_tiles):
                return
            tg_, ts_, w_, off_ = exec_tiles[i]
            t = xpool.tile([128, KD * w_], bf16, tag="xt", bufs=3)
            xt_sbs[i] = t
            if i == 0:
                # head: split into 4 k-chunk pairs on the scalar HWDGE so
                # the gate m0 k-loop can start on k0-1 while k2-7 stream.
                for ci in range(4):
                    ks, ke = 2 * ci, 2 * ci + 2
                    nc.scalar.dma_start(
                        t[:, ks * w_ : ke * w_],
                        xt_d.ap()[:, KD * off_ + ks * w_ : KD * off_ + ke * w_],
                    )
            else:
                # Act ring, issued between silu sections: the in-order ACT
                # queue + FIFO ring naturally pace the transfer behind the
                # wu blocks, keeping these bytes off the contended head.
                nc.scalar.dma_start(
                    t[:], xt_d.ap()[:, KD * off_ : KD * (off_ + w_)]
                )

        ti_global = 0
        for gi in order:
            g = groups[gi]
            wg_d, wu_d, wd_d = w_ds[gi]
            first_group = gi == order[0]
            wg_blk, wu_blk, wd_blk = [], [], []
            for bi, (s, e) in enumerate(GU_BLKS):
                span = (e - s) * GU_M
                tg = wpool.tile([128, span], bf16, tag=f"wg_b{bi}")
                nc.sync.dma_start(tg[:], wg_d.ap()[:, s * GU_M : e * GU_M])
                wg_blk.append(tg)
            if first_group:
                make_xt(0)
            for bi, (s, e) in enumerate(GU_BLKS):
                span = (e - s) * GU_M
                tu = wpool.tile([128, span], bf16, tag=f"wu_b{bi}")
                nc.gpsimd.dma_start(tu[:], wu_d.ap()[:, s * GU_M : e * GU_M])
                wu_blk.append(tu)
            for bi, (s, e) in enumerate(WD_BLKS):
                span = (e - s) * D_M
                td = wpool.tile([128, span], bf16, tag=f"wd_b{bi}")
                nc.sync.dma_start(td[:], wd_d.ap()[:, s * D_M : e * D_M])
                wd_blk.append(td)

            def gu_slice(blk_list, blks, m, k):
                for bi, (s, e) in enumerate(blks):
                    if s <= m < e:
                        base = ((m - s) * KD + k) * 128
                        return blk_list[bi][:, base : base + 128]
                raise AssertionError

            def wd_slice(m2, k2):
                for bi, (s, e) in enumerate(WD_BLKS):
                    if s <= m2 < e:
                        base = ((m2 - s) * KF + k2) * 128
                        return wd_blk[bi][:, base : base + 128]
                raise AssertionError

            group_tiles = [t for t in tiles if t[0] == g]
            for ti, (tg_, ts, w, off) in enumerate(group_tiles):
                final_tile = ti_global == len(exec_tiles) - 1
                make_xt(ti_global)          # no-op if prefetched
                xt_sb = xt_sbs[ti_global]
                rhs = lambda k, t=xt_sb, w_=w: t[:, k * w_ : k * w_ + w_]

                # Interleaved gate/up per m: each weight stream is consumed
                # at ~75 GB/s, matching what its ring can deliver while the
                # head is contended by all 8 cores.
                h_sb = hpool.tile([128, KF, 512], bf16, tag="h", bufs=2)
                for m in range(KF):
                    ps_g = psg.tile([128, 512], f32, tag="psg")
                    ps_u = psu.tile([128, 512], f32, tag="psu")
                    for k in range(KD):
                        nc.tensor.matmul(
                            ps_g[:, :w],
                            gu_slice(wg_blk, GU_BLKS, m, k),
                            rhs(k),
                            start=(k == 0),
                            stop=(k == KD - 1),
                        )
                    for k in range(KD):
                        nc.tensor.matmul(
                            ps_u[:, :w],
                            gu_slice(wu_blk, GU_BLKS, m, k),
                            rhs(k),
                            start=(k == 0),
                            stop=(k == KD - 1),
                        )
                    g_sb = gpool.tile([128, 512], bf16, tag="gact", bufs=3)
                    nc.scalar.activation(
                        g_sb[:, :w],
                        ps_g[:, :w],
                        mybir.ActivationFunctionType.Silu,
                    )
                    nc.vector.tensor_mul(h_sb[:, m, :w], g_sb[:, :w], ps_u[:, :w])
                # Prefetch the next tile's tokens here: the in-order ACT
                # queue reaches this issue only after this tile's last silu,
                # pacing the transfer well off the contended head while
                # still landing a full down-pass (~27 us) ahead of use.
                make_xt(ti_global + 1)
                # Down pass.  Output: one scalar DMA per tile, except the
                # final tile which keeps per-m2 scalar DMAs so the last
                # bytes leave as soon as computed.
                o_sb = opool.tile([128, KD * w], bf16, tag="o", bufs=2)
                for m2 in range(KD):
                    ps_d = psd.tile([128, 512], f32, tag="psd")
                    for k2 in range(KF):
                        nc.tensor.matmul(
                            ps_d[:, :w],
                            wd_slice(m2, k2),
                            h_sb[:, k2, :w],
                            start=(k2 == 0),
                            stop=(k2 == KF - 1),
                        )
                    nc.vector.tensor_copy(o_sb[:, m2 * w : (m2 + 1) * w], ps_d[:, :w])
                    if final_tile:
                        nc.scalar.dma_start(
                            out_d.ap()[
                                :, KD * off + m2 * w : KD * off + (m2 + 1) * w
                            ],
                            o_sb[:, m2 * w : (m2 + 1) * w],
                        )
                if not final_tile:
                    nc.scalar.dma_start(
                        out_d.ap()[:, KD * off : KD * (off + w)], o_sb[:]
                    )
                ti_global += 1

    nc.compile()
    return nc


def kernel(hidden_states, router_w, router_b, wg, wu, wd, top_k):
    hidden_states = np.asarray(hidden_states, dtype=np.float32)
    router_w = np.asarray(router_w, dtype=np.float32)
    router_b = np.asarray(router_b, dtype=np.float32)
    wg = np.asarray(wg, dtype=np.float32)
    wu = np.asarray(wu, dtype=np.float32)
    wd = np.asarray(wd, dtype=np.float32)

    Bb, S, Dd = hidden_states.shape
    x = hidden_states.reshape(-1, Dd)
    assert x.shape == (T, D)

    topi, topw = _route(x, router_w, router_b, top_k)
    per_expert = {}
    for e in range(1, E):
        sel = np.nonzero((topi == (e - 1)).any(axis=1))[0]
        w = topw[sel][topi[sel] == (e - 1)]
        per_expert[e] = (sel.astype(np.int64), w.astype(np.float32))

    caps, slots = _plan_slots(per_expert)
    tiles, total = _tiles_for(caps)

    # Per-expert weight layouts (bf16, m-tile-major); computed once per expert.
    experts_used = sorted({s[0] for core in slots for s in core})
    wg_l = {e: _wgu_layout(wg[e]) for e in experts_used}
    wu_l = {e: _wgu_layout(wu[e]) for e in experts_used}
    wd_l = {e: _wd_layout(wd[e]) for e in experts_used}

    groups = [g for g in range(len(caps)) if caps[g] > 0]
    in_maps = []
    for c in range(NCORES):
        xt_flat = np.zeros((128, KD * total), dtype=_BF16)
        for tg_, ts, w, off in tiles:
            idx = slots[c][tg_][1]
            seg = idx[ts : ts + w]
            gx = np.zeros((w, D), dtype=np.float32)
            gx[: len(seg)] = x[seg]
            blk = gx.T.reshape(KD, 128, w).transpose(1, 0, 2).reshape(128, KD * w)
            xt_flat[:, KD * off : KD * (off + w)] = blk.astype(_BF16)
        m = {"xt": xt_flat}
        for g in groups:
            e = slots[c][g][0]
            m[f"w{g}g"] = wg_l[e]
            m[f"w{g}u"] = wu_l[e]
            m[f"w{g}d"] = wd_l[e]
        in_maps.append(m)

    if caps not in _COMPILED:
        _COMPILED[caps] = _build_graph(caps)
    nc = _COMPILED[caps]

    # If the environment lacks antenv.axon_hooks, running with BASS_TRACE=1
    # would crash inside run_bass_kernel_spmd on an unguarded import; provide
    # an inert hook registry so tracing degrades to a warning instead.
    try:
        import antenv.axon_hooks  # noqa: F401
    except Exception:
        import sys as _sys
        import types as _types

        _m = _types.ModuleType("antenv.axon_hooks")
        _m._h = None
        _m.set_axon_ntff_profile_hook = lambda h: setattr(_m, "_h", h)
        _m.get_axon_ntff_profile_hook = lambda: getattr(_m, "_h", None)
        _sys.modules["antenv.axon_hooks"] = _m

    from concourse.bass_utils import run_bass_kernel_spmd

    res = run_bass_kernel_spmd(nc, in_maps, core_ids=list(range(NCORES)))
    global LAST_EXEC_NS, LAST_RESULT
    LAST_EXEC_NS = res.exec_time_ns
    LAST_RESULT = res

    out = np.zeros((T, D), dtype=np.float32)
    for c in range(NCORES):
        yT = res.results[c]["out"]  # [128, KD*total] f32
        for tg_, ts, w, off in tiles:
            e, idx, wt = slots[c][tg_]
            seg = idx[ts : ts + w]
            wseg = wt[ts : ts + w]
            if len(seg) == 0:
                continue
            y = (
                yT[:, KD * off : KD * (off + w)]
                .astype(np.float32)
                .reshape(128, KD, w)
                .transpose(1, 0, 2)
                .reshape(D, w)
                .T
            )
            out[seg] += wseg[:, None] * y[: len(seg)]

    return out.reshape(Bb, S, D)



# revision 14
# speedup vs baseline: 1.0052x; 1.0052x over previous
"""Always-on MoE forward (expert 0 dense + top-k of 7 routed) on 8 TRN2 cores.

Strategy
--------
The router (4096x1024 @ 1024x7 matmul + softmax + top-2) is ~58 MFLOP --
negligible -- so it runs on host in numpy as part of computing the sharding
plan.  The expensive part (expert SwiGLU MLPs, ~155 GFLOP with top-2
sparsity) runs on device, expert-parallel with host-side token
dispatch/combine:

- SPMD graph: every core runs TWO or THREE weight groups with compile-time
  capacities.  Each (core, group) slot is filled with tokens of ONE expert
  (weights supplied per-core via in_maps).  A capacity search (2-group
  legacy + 3-group DFS assignment where an expert may span several slots on
  different cores) picks the group sizes minimizing the modeled PE-stream
  time; with 3 groups the per-core capacity lands within ~1% of the ideal
  (T + k*T) / 8 balance point (e.g. caps (344, 412, 792) -> 1548 vs 1536
  ideal, vs 1586 for the best 2-group plan).  Leftover slots are filled
  with always-on expert-0 tokens.
- Host gathers each core's tokens (transposed k-chunk layout, bf16), device
  computes down(silu(x@wg) * (x@wu)) for both groups, host scatter-adds the
  outputs with the combine weights (expert-0 weight 1.0).

Device kernel: pure dense matmul streaming, weights stationary in SBUF per
group.  All DRAM views are pre-tiled on host into [128, flat] layouts so
every DMA is contiguous per partition and every matmul operand is a direct
SBUF slice; weights are DMA'd per 128-wide m-tile (0.25 MB) so the first
matmuls start ~8 us into the kernel instead of waiting for full weight sets,
and a PE pre-warm burst flips the HAM clock gate during the DMA ramp.
"""

import numpy as np
import ml_dtypes

D = 1024
DFF = 2048
E = 8
NCORES = 8
T = 2 * 2048  # B * S
KD = D // 128    # contraction chunks over D
KF = DFF // 128  # contraction chunks over DFF

_COMPILED = {}

_BF16 = ml_dtypes.bfloat16

LAST_EXEC_NS = None
LAST_RESULT = None


def _route(x, router_w, router_b, top_k):
    """Replicates the reference router in numpy f32: returns (topi, topw)."""
    logits = x.astype(np.float32) @ router_w.astype(np.float32) + router_b.astype(
        np.float32
    )
    m = logits.max(axis=-1, keepdims=True)
    p = np.exp(logits - m)
    p /= p.sum(axis=-1, keepdims=True)
    k = int(top_k)
    topi = np.argpartition(-p, kth=k - 1, axis=-1)[:, :k]  # top-k set (unordered)
    topw = np.take_along_axis(p, topi, axis=-1)
    topw = topw / topw.sum(axis=-1, keepdims=True)
    return topi, topw.astype(np.float32)


def _split_even(idx, w, n):
    """Split (idx, w) into n near-equal chunks."""
    c = len(idx)
    sizes = [(c + n - 1 - i) // n for i in range(n)]
    out, pos = [], 0
    for s in sizes:
        out.append((idx[pos : pos + s], w[pos : pos + s]))
        pos += s
    return out


def _widths_for(cap):
    """Tile widths for a group capacity.

    First tile as wide as possible (512) -- during the first tile the weight
    stream runs near the HBM rate, and a wider tile lowers the per-ns weight
    demand.  Remaining capacity in near-equal tiles, kept above the ~280
    matmul instruction floor (LDWEIGHTS + dispatch) when possible.
    """
    if cap <= 0:
        return []

    def near_equal(c, n):
        return [(c + n - 1 - i) // n for i in range(n)]

    plain = near_equal(cap, -(-cap // 512))
    if cap > 512:
        rem = cap - 512
        lead = [512] + near_equal(rem, -(-rem // 512))
    else:
        lead = plain
    cost = lambda ws: sum(max(w, 280) for w in ws)
    return lead if cost(lead) <= cost(plain) else plain


def _tile_cost(cap):
    """Modeled PE-stream ns for one group: 384 matmuls per tile, each
    max(streaming, LDWEIGHTS-floor ~108ns)."""
    return sum(384 * max(w / 2.4 + 2.5, 108.0) for w in _widths_for(cap))


def _assign_k3(caps, counts_list):
    """DFS: assign each routed expert a (i0,i1,i2) slot combo (i_g slots of
    group g); e0 must fit in the leftover slots.  Cost is fixed by `caps`,
    so the FIRST feasible assignment wins (min-capacity-first ordering keeps
    leftover for e0 large).  Returns {expert: (i0,i1,i2)} or None."""
    sizes = caps
    total_cap = NCORES * sum(sizes)
    order = sorted(counts_list, key=lambda ec: -ec[1])
    combos_per = []
    for e, c in order:
        opts = []
        for i0 in range(4):
            for i1 in range(4):
                for i2 in range(4):
                    capx = i0 * sizes[0] + i1 * sizes[1] + i2 * sizes[2]
                    if capx >= c:
                        opts.append((capx, (i0, i1, i2)))
        opts.sort()
        if not opts:
            return None
        combos_per.append((e, c, opts[:10]))
    # lower bound on remaining assigned capacity from expert idx onward
    lb = [0] * (len(combos_per) + 1)
    for i in range(len(combos_per) - 1, -1, -1):
        lb[i] = lb[i + 1] + combos_per[i][2][0][0]
    found = [None]
    budget = [60000]
    def dfs(idx, use, tot, picks):
        if found[0] is not None or budget[0] <= 0:
            return
        budget[0] -= 1
        if total_cap - tot - lb[idx] < T:
            return  # e0 can no longer fit
        if idx == len(combos_per):
            found[0] = dict(picks)
            return
        e, c, opts = combos_per[idx]
        for capx, inc in opts:
            nu = [u + i for u, i in zip(use, inc)]
            if any(u > NCORES for u in nu):
                continue
            picks.append((e, inc))
            dfs(idx + 1, nu, tot + capx, picks)
            picks.pop()
            if found[0] is not None:
                return
    dfs(0, [0, 0, 0], 0, [])
    return found[0]


def _plan_k2(counts):
    """Legacy 2-group plan: (cost, caps, combos) -- each routed expert gets
    j B-slots; e0 fills the rest."""
    cands = sorted(
        {-(-c // j) for c in counts.values() for j in range(1, 9) if c} | {512}
    )
    best = None
    for Bc in cands:
        npieces = sum(-(-c // Bc) for c in counts.values() if c)
        if npieces > NCORES:
            continue
        nfree = NCORES - npieces
        e0_in_b = min(T, nfree * Bc)
        A = -(-(T - e0_in_b) // NCORES) if e0_in_b < T else 0
        tot = _tile_cost(A) + _tile_cost(Bc)
        if best is None or tot < best[0]:
            best = (tot, A, Bc)
    tot, A, B = best
    combos = {e: (0, -(-c // B)) for e, c in counts.items() if c}
    return tot, (A, B), combos


def _plan_k3(counts):
    """Search 3-group capacities; returns (cost, caps, combos) or None."""
    from functools import lru_cache

    tc = lru_cache(maxsize=None)(_tile_cost)
    lo = -(-(T + sum(counts.values())) // NCORES)
    counts_list = [(e, c) for e, c in counts.items() if c > 0]
    cands = []
    for C in range(560, 1241, 4):
        cC = tc(C)
        for B in range(240, C + 1, 4):
            cB = tc(B)
            a0 = max(160, lo - B - C)
            a1 = min(B, lo + 52 - B - C)
            a0 += (-a0) % 4
            for A in range(a0, a1 + 1, 4):
                cands.append((tc(A) + cB + cC, A, B, C))
    cands.sort()
    tried = 0
    for cost, A, B, C in cands:
        tried += 1
        if tried > 25000:
            break
        picks = _assign_k3((A, B, C), counts_list)
        if picks is not None:
            return cost, (A, B, C), picks
    return None


def _plan_slots(per_expert):
    """Pick group capacities and fill the 8 slots per group.

    per_expert: {e: (idx, w)} for routed experts.
    Returns (caps, slots); slots[c] = tuple over groups of (e, idx, w).
    """
    counts = {e: len(v[0]) for e, v in per_expert.items()}
    cost2, caps2, combos2 = _plan_k2(counts)
    plan3 = _plan_k3(counts)
    if plan3 is not None and plan3[0] < cost2 - 2000:
        cost, caps, combos = plan3
    else:
        cost, caps, combos = cost2, caps2, combos2

    ngroups = len(caps)
    ones = np.ones(T, dtype=np.float32)
    e0_idx = np.arange(T, dtype=np.int64)

    # Build expert pieces per group: fill assigned slots, remainder last.
    group_pieces = [[] for _ in range(ngroups)]
    for e in sorted(counts, key=lambda e: -counts[e]):
        if counts[e] == 0:
            continue
        idx, w = per_expert[e]
        combo = combos[e]
        slot_sizes = []
        for g in range(ngroups):
            slot_sizes += [(g, caps[g])] * combo[g]
        # fill largest slots first so at most one slot is partial
        slot_sizes.sort(key=lambda gs: -gs[1])
        pos = 0
        for g, size in slot_sizes:
            take = min(size, len(idx) - pos)
            group_pieces[g].append((e, idx[pos : pos + take], w[pos : pos + take]))
            pos += take
        assert pos >= len(idx), f"expert {e} tokens not covered"
    # e0 fills remaining slots (largest first so at most one is partial)
    free = []
    for g in range(ngroups):
        free += [(g, caps[g])] * (NCORES - len(group_pieces[g]))
    free.sort(key=lambda gs: -gs[1])
    pos = 0
    for g, size in free:
        take = min(size, T - pos)
        group_pieces[g].append((0, e0_idx[pos : pos + take], ones[pos : pos + take]))
        pos += take
    assert pos >= T, "expert-0 tokens not covered"

    slots = [
        tuple(group_pieces[g][c] for g in range(ngroups)) for c in range(NCORES)
    ]
    return caps, slots


def _wgu_layout(w2d):
    """[D, DFF] f32 -> [128, KF*KD*128] bf16 m-tile-major layout:
    element [p, (m*KD + k)*128 + c] = W[k*128 + p, m*128 + c]."""
    a = w2d.reshape(KD, 128, KF, 128).transpose(1, 2, 0, 3).reshape(128, -1)
    return np.ascontiguousarray(a).astype(_BF16)


def _wd_layout(w2d):
    """[DFF, D] f32 -> [128, KD*KF*128] bf16 m-tile-major layout:
    element [p, (m*KF + k)*128 + c] = W[k*128 + p, m*128 + c]."""
    a = w2d.reshape(KF, 128, KD, 128).transpose(1, 2, 0, 3).reshape(128, -1)
    return np.ascontiguousarray(a).astype(_BF16)


def _tiles_for(caps):
    """Token tiles [(group, start_in_group, width, flat_offset)] per core."""
    tiles = []
    off = 0
    for g, cap in enumerate(caps):
        ts = 0
        for w in _widths_for(cap):
            tiles.append((g, ts, w, off))
            off += w
            ts += w
    return tiles, off


def _build_graph(caps):
    import concourse.mybir as mybir
    import concourse.tile as tile
    from concourse import bacc
    from contextlib import ExitStack

    bf16 = mybir.dt.bfloat16
    f32 = mybir.dt.float32

    tiles, total = _tiles_for(caps)
    groups = [g for g in range(len(caps)) if caps[g] > 0]

    nc = bacc.Bacc("TRN2", target_bir_lowering=False)

    xt_d = nc.declare_dram_parameter("xt", [128, KD * total], bf16, isOutput=False)
    w_ds = []
    for g in groups:
        w_ds.append(
            (
                nc.declare_dram_parameter(
                    f"w{g}g", [128, KF * KD * 128], bf16, isOutput=False
                ),
                nc.declare_dram_parameter(
                    f"w{g}u", [128, KF * KD * 128], bf16, isOutput=False
                ),
                nc.declare_dram_parameter(
                    f"w{g}d", [128, KD * KF * 128], bf16, isOutput=False
                ),
            )
        )
    out_d = nc.declare_dram_parameter("out", [128, KD * total], bf16, isOutput=True)

    with tile.TileContext(nc) as tc, ExitStack() as ctx:
        # One SBUF pool + one PSUM pool (per-tag bufs) -- the end-of-kernel
        # semaphore drain scales with pool count, so fewer pools shorten the
        # epilogue.
        sbp = ctx.enter_context(tc.tile_pool(name="sb", bufs=1))
        wpool = xpool = hpool = gpool = opool = sbp
        psp = ctx.enter_context(tc.tile_pool(name="ps", bufs=2, space="PSUM"))
        psd = psp

        GU_M = KD * 128  # bytes-per-m-tile span (elems) for wg/wu
        D_M = KF * 128   # for wd

        # PE pre-warm: a burst of dummy matmuls that (a) bridges the gap
        # until the first weight/token chunks land (~11 us: preamble ~7.4
        # + DMA cold latency under 8-core HBM contention) and (b)
        # accumulates the ~3.4 us of sustained PE busy the HAM clock gate
        # needs to flip 4/8 -> 8/8, so the first REAL matmuls run at
        # 2.4 GHz instead of 1.2.
        warm_sb = wpool.tile([128, 128], bf16, tag="warm")
        nc.gpsimd.memset(warm_sb[:], 0)
        ps_w = psp.tile([128, 128], f32, tag="psw", bufs=1)
        for _ in range(38):
            nc.tensor.matmul(ps_w[:], warm_sb[:], warm_sb[:], start=True, stop=True)

        # Larger group first: its weights stream in unblocked at t=0, and the
        # other group's weight reloads (WAR-gated on this group's last use of
        # each m-tile slot) get a long compute window to hide under.
        order = sorted(range(len(groups)), key=lambda gi: -caps[groups[gi]])

        # Weight DMA blocks: geometric doubling gives a tiny first block
        # (first matmul starts early) and few total DMAs (the HWDGE issue
        # path allows only ~6 outstanding dma_starts per engine).  wg+wd
        # ride sync; xt + wu + outs ride scalar (with xt head chunks split
        # so the first matmul needs only ~0.25 MB); gpsimd SWDGE carries
        # only the head xt k4-7 chunks (its ~2-3 us per-DMA cost is too
        # slow for anything bandwidth-critical).
        GU_BLKS = [(0, 1), (1, 2), (2, 4), (4, 8), (8, 16)]   # m-tile ranges
        WU_BLKS = [(0, 1), (1, 3), (3, 8), (8, 16)]           # m-tile ranges
        WD_BLKS = [(0, 2), (2, 4), (4, 8)]                    # m2-tile ranges

        # Execution chunks: tiles of one group processed together.  Tiles
        # within a group are PAIRED: the gate/up m-loop interleaves both
        # tiles over one pass of the weight stream, so each weight m-tile
        # is amortized over the pair's combined width.  For the head group
        # (e.g. 512+280) this cuts the weight demand rate from ~145 GB/s
        # to ~97 GB/s during the HBM-contended ramp -- the dominant source
        # of PE idle in the single-tile schedule.
        chunks = []  # (gi, [tile, ...]) with 1-2 tiles each
        for gi in order:
            g = groups[gi]
            gt = [t for t in tiles if t[0] == g]
            for i in range(0, len(gt), 2):
                chunks.append((gi, gt[i : i + 2]))
        flat_tiles = [t for _, pr in chunks for t in pr]

        # xt SBUF tiles; bufs=3 paces the prefetch (tile i+3's DMA
        # WAR-waits on tile i's last read).
        xt_sbs = {}

        def make_xt(i, head=False):
            if i in xt_sbs or i >= len(flat_tiles):
                return
            tg_, ts_, w_, off_ = flat_tiles[i]
            t = xpool.tile([128, KD * w_], bf16, tag="xt", bufs=3)
            xt_sbs[i] = t
            if not head:
                nc.scalar.dma_start(
                    t[:], xt_d.ap()[:, KD * off_ : KD * (off_ + w_)]
                )

        def head_xt_chunk(i, ci):
            # ci 0-1: k01 / k23 on scalar HWDGE (fast first byte);
            # ci 2: one k4-7 chunk on gpsimd SWDGE (lands ~12 us, needed
            # ~14.5; one big chunk because SWDGE pays ~2 us per DMA).
            t = xt_sbs[i]
            tg_, ts_, w_, off_ = flat_tiles[i]
            if ci < 2:
                ks, ke = 2 * ci, 2 * ci + 2
                nc.scalar.dma_start(
                    t[:, ks * w_ : ke * w_],
                    xt_d.ap()[:, KD * off_ + ks * w_ : KD * off_ + ke * w_],
                )
            else:
                nc.gpsimd.dma_start(
                    t[:, 4 * w_ : 8 * w_],
                    xt_d.ap()[:, KD * off_ + 4 * w_ : KD * off_ + 8 * w_],
                )

        group_state = {}

        def issue_wu(gi):
            # wu blocks ride the scalar ring.  For groups after the first
            # this is called at the END of the previous group's gate/up
            # m-loop: the reload's WAR deps (last tile's up m reads) are
            # long cleared, and the Act ring transfers it during the
            # previous group's down passes, well ahead of first use.
            wu_d = w_ds[gi][1]
            blks = []
            for bi, (s, e) in enumerate(WU_BLKS):
                span = (e - s) * GU_M
                tu = wpool.tile([128, span], bf16, tag=f"wu_b{bi}")
                nc.scalar.dma_start(tu[:], wu_d.ap()[:, s * GU_M : e * GU_M])
                blks.append(tu)
            group_state.setdefault(gi, {})["wu"] = blks

        def issue_wg_wd(gi):
            wg_d, _, wd_d = w_ds[gi]
            st = group_state.setdefault(gi, {})
            blks = []
            for bi, (s, e) in enumerate(GU_BLKS):
                span = (e - s) * GU_M
                tg = wpool.tile([128, span], bf16, tag=f"wg_b{bi}")
                nc.sync.dma_start(tg[:], wg_d.ap()[:, s * GU_M : e * GU_M])
                blks.append(tg)
            st["wg"] = blks
            blks = []
            for bi, (s, e) in enumerate(WD_BLKS):
                span = (e - s) * D_M
                td = wpool.tile([128, span], bf16, tag=f"wd_b{bi}")
                nc.sync.dma_start(td[:], wd_d.ap()[:, s * D_M : e * D_M])
                blks.append(td)
            st["wd"] = blks

        def gu_slice(blk_list, blks, m, k):
            for bi, (s, e) in enumerate(blks):
                if s <= m < e:
                    base = ((m - s) * KD + k) * 128
                    return blk_list[bi][:, base : base + 128]
            raise AssertionError

        def wd_slice(blk_list, m2, k2):
            for bi, (s, e) in enumerate(WD_BLKS):
                if s <= m2 < e:
                    base = ((m2 - s) * KF + k2) * 128
                    return blk_list[bi][:, base : base + 128]
            raise AssertionError

        ti_flat = 0
        seen_groups = set()
        for ci_, (gi, pair) in enumerate(chunks):
            g = groups[gi]
            if gi not in seen_groups:
                seen_groups.add(gi)
                issue_wg_wd(gi)
                if ci_ == 0:
                    # Head: interleave the pair's xt k-chunks in demand
                    # order on the scalar ring, then the wu lead blocks.
                    for i in range(len(pair)):
                        make_xt(ti_flat + i, head=True)
                    for i in range(len(pair)):
                        head_xt_chunk(ti_flat + i, 0)
                        head_xt_chunk(ti_flat + i, 1)
                    issue_wu(gi)
                    for i in range(len(pair)):
                        head_xt_chunk(ti_flat + i, 2)
            st = group_state[gi]
            wg_blk, wu_blk, wd_blk = st["wg"], st["wu"], st["wd"]

            for i in range(len(pair)):
                make_xt(ti_flat + i)  # no-op if prefetched
            rhs_f = []
            for i, (tg_, ts, w, off) in enumerate(pair):
                t = xt_sbs[ti_flat + i]
                rhs_f.append(lambda k, t=t, w_=w: t[:, k * w_ : k * w_ + w_])
            h_tiles = [
                hpool.tile([128, KF, 512], bf16, tag="h", bufs=2, name="h_sb")
                for _ in pair
            ]

            # Interleaved gate/up over the pair; 4 PSUM banks (psg0/1,
            # psu0/1, bufs=1 each) so a pair's gate+up groups are all
            # in flight; solo chunks alternate the same banks by m parity.
            for m in range(KF):
                ps_gs, ps_us = [], []
                for i, (tg_, ts, w, off) in enumerate(pair):
                    sel = i if len(pair) > 1 else m % 2
                    ps_g = psp.tile([128, 512], f32, tag=f"psg{sel}", bufs=1)
                    ps_gs.append(ps_g)
                    for k in range(KD):
                        nc.tensor.matmul(
                            ps_g[:, :w],
                            gu_slice(wg_blk, GU_BLKS, m, k),
                            rhs_f[i](k),
                            start=(k == 0),
                            stop=(k == KD - 1),
                        )
                for i, (tg_, ts, w, off) in enumerate(pair):
                    sel = i if len(pair) > 1 else m % 2
                    ps_u = psp.tile([128, 512], f32, tag=f"psu{sel}", bufs=1)
                    ps_us.append(ps_u)
                    for k in range(KD):
                        nc.tensor.matmul(
                            ps_u[:, :w],
                            gu_slice(wu_blk, WU_BLKS, m, k),
                            rhs_f[i](k),
                            start=(k == 0),
                            stop=(k == KD - 1),
                        )
                for i, (tg_, ts, w, off) in enumerate(pair):
                    g_sb = gpool.tile([128, 512], bf16, tag="gact", bufs=4)
                    nc.scalar.activation(
                        g_sb[:, :w],
                        ps_gs[i][:, :w],
                        mybir.ActivationFunctionType.Silu,
                    )
                    nc.vector.tensor_mul(
                        h_tiles[i][:, m, :w], g_sb[:, :w], ps_us[i][:, :w]
                    )

            # Prefetch the next chunk's tokens and (at a group boundary)
            # the next group's wu blocks: the in-order Act queue reaches
            # these issues right after this chunk's last silu, so the
            # transfers ride during the down passes, ahead of use.
            nxt = ti_flat + len(pair)
            for i in range(len(chunks[ci_ + 1][1]) if ci_ + 1 < len(chunks) else 0):
                make_xt(nxt + i)
            if ci_ + 1 < len(chunks):
                ngi = chunks[ci_ + 1][0]
                if ngi not in group_state or "wu" not in group_state[ngi]:
                    issue_wu(ngi)

            # Down passes, serial over the pair.  Output: one DMA per
            # tile, except the final tile which keeps per-m2 DMAs -- and a
            # split final m2 -- so the last bytes leave as soon as computed.
            for i, (tg_, ts, w, off) in enumerate(pair):
                final_tile = ti_flat + i == len(flat_tiles) - 1
                h_sb = h_tiles[i]
                o_sb = opool.tile([128, KD * w], bf16, tag="o", bufs=2)
                for m2 in range(KD):
                    if final_tile and m2 == KD - 1:
                        # split the last m2 column-wise: cast+DMA of the
                        # first half overlaps the second half's matmuls.
                        hw1 = (w // 2 + 3) & ~3
                        for cs, cw in ((0, hw1), (hw1, w - hw1)):
                            ps_d = psd.tile([128, 512], f32, tag="psd")
                            for k2 in range(KF):
                                nc.tensor.matmul(
                                    ps_d[:, :cw],
                                    wd_slice(wd_blk, m2, k2),
                                    h_sb[:, k2, cs : cs + cw],
                                    start=(k2 == 0),
                                    stop=(k2 == KF - 1),
                                )
                            nc.vector.tensor_copy(
                                o_sb[:, m2 * w + cs : m2 * w + cs + cw],
                                ps_d[:, :cw],
                            )
                            nc.scalar.dma_start(
                                out_d.ap()[
                                    :,
                                    KD * off + m2 * w + cs : KD * off
                                    + m2 * w
                                    + cs
                                    + cw,
                                ],
                                o_sb[:, m2 * w + cs : m2 * w + cs + cw],
                            )
                        continue
                    ps_d = psd.tile([128, 512], f32, tag="psd")
                    for k2 in range(KF):
                        nc.tensor.matmul(
                            ps_d[:, :w],
                            wd_slice(wd_blk, m2, k2),
                            h_sb[:, k2, :w],
                            start=(k2 == 0),
                            stop=(k2 == KF - 1),
                        )
                    nc.vector.tensor_copy(o_sb[:, m2 * w : (m2 + 1) * w], ps_d[:, :w])
                    if final_tile:
                        nc.scalar.dma_start(
                            out_d.ap()[
                                :, KD * off + m2 * w : KD * off + (m2 + 1) * w
                            ],
                            o_sb[:, m2 * w : (m2 + 1) * w],
                        )
                if not final_tile:
                    nc.scalar.dma_start(
                        out_d.ap()[:, KD * off : KD * (off + w)], o_sb[:]
                    )
            ti_flat += len(pair)

    nc.compile()
    return nc


def kernel(hidden_states, router_w, router_b, wg, wu, wd, top_k):
    hidden_states = np.asarray(hidden_states, dtype=np.float32)
    router_w = np.asarray(router_w, dtype=np.float32)
    router_b = np.asarray(router_b, dtype=np.float32)
    wg = np.asarray(wg, dtype=np.float32)
    wu = np.asarray(wu, dtype=np.float32)
    wd = np.asarray(wd, dtype=np.float32)

    Bb, S, Dd = hidden_states.shape
    x = hidden_states.reshape(-1, Dd)
    assert x.shape == (T, D)

    topi, topw = _route(x, router_w, router_b, top_k)
    per_expert = {}
    for e in range(1, E):
        sel = np.nonzero((topi == (e - 1)).any(axis=1))[0]
        w = topw[sel][topi[sel] == (e - 1)]
        per_expert[e] = (sel.astype(np.int64), w.astype(np.float32))

    caps, slots = _plan_slots(per_expert)
    tiles, total = _tiles_for(caps)

    # Per-expert weight layouts (bf16, m-tile-major); computed once per expert.
    experts_used = sorted({s[0] for core in slots for s in core})
    wg_l = {e: _wgu_layout(wg[e]) for e in experts_used}
    wu_l = {e: _wgu_layout(wu[e]) for e in experts_used}
    wd_l = {e: _wd_layout(wd[e]) for e in experts_used}

    groups = [g for g in range(len(caps)) if caps[g] > 0]
    in_maps = []
    for c in range(NCORES):
        xt_flat = np.zeros((128, KD * total), dtype=_BF16)
        for tg_, ts, w, off in tiles:
            idx = slots[c][tg_][1]
            seg = idx[ts : ts + w]
            gx = np.zeros((w, D), dtype=np.float32)
            gx[: len(seg)] = x[seg]
            blk = gx.T.reshape(KD, 128, w).transpose(1, 0, 2).reshape(128, KD * w)
            xt_flat[:, KD * off : KD * (off + w)] = blk.astype(_BF16)
        m = {"xt": xt_flat}
        for g in groups:
            e = slots[c][g][0]
            m[f"w{g}g"] = wg_l[e]
            m[f"w{g}u"] = wu_l[e]
            m[f"w{g}d"] = wd_l[e]
        in_maps.append(m)

    if caps not in _COMPILED:
        _COMPILED[caps] = _build_graph(caps)
    nc = _COMPILED[caps]

    # If the environment lacks antenv.axon_hooks, running with BASS_TRACE=1
    # would crash inside run_bass_kernel_spmd on an unguarded import; provide
    # an inert hook registry so tracing degrades to a warning instead.
    try:
        import antenv.axon_hooks  # noqa: F401
    except Exception:
        import sys as _sys
        import types as _types

        _m = _types.ModuleType("antenv.axon_hooks")
        _m._h = None
        _m.set_axon_ntff_profile_hook = lambda h: setattr(_m, "_h", h)
        _m.get_axon_ntff_profile_hook = lambda: getattr(_m, "_h", None)
        _sys.modules["antenv.axon_hooks"] = _m

    from concourse.bass_utils import run_bass_kernel_spmd

    res = run_bass_kernel_spmd(nc, in_maps, core_ids=list(range(NCORES)))
    global LAST_EXEC_NS, LAST_RESULT
    LAST_EXEC_NS = res.exec_time_ns
    LAST_RESULT = res

    out = np.zeros((T, D), dtype=np.float32)
    for c in range(NCORES):
        yT = res.results[c]["out"]  # [128, KD*total] f32
        for tg_, ts, w, off in tiles:
            e, idx, wt = slots[c][tg_]
            seg = idx[ts : ts + w]
            wseg = wt[ts : ts + w]
            if len(seg) == 0:
                continue
            y = (
                yT[:, KD * off : KD * (off + w)]
                .astype(np.float32)
                .reshape(128, KD, w)
                .transpose(1, 0, 2)
                .reshape(D, w)
                .T
            )
            out[seg] += wseg[:, None] * y[: len(seg)]

    return out.reshape(Bb, S, D)

